# revision 2
# baseline (speedup 1.0000x reference)
"""Trainium2 Bass kernel for per-sample channel-modulated 3x3 conv (CoModConv).

Math (matches the reference nn.Module):
    s = lrelu(lrelu(lrelu(y @ w0.T + b0) @ w1.T + b1) @ w2.T + b2)   # (B, C_in)
    out = conv3x3(x * s[:, :, None, None], conv_w, pad=1)            # (B, C_out, H, W)

Strategy: data-parallel over batch, 2 samples per NeuronCore (8 cores),
with 1D Winograd F(2,3) along H (direct along W). Per output row-pair:
    even = M0 + 0.5*(M1 + M2)        odd = 0.5*(M1 - M2) - M3
where M_a = conv_w-row-combos (U_a) applied to row-combos of x (V_a).
The 0.5 and the per-sample channel modulation s both fold into the
stationary U tiles, so the tensor engine does 4 comps x 3 kj taps x 2
ci-tiles = 24 matmuls per 16 output rows instead of direct conv's 36:
a 1.5x reduction in PE work (the cost-model bottleneck).
  - V comps (d0-d2, d1+d2, d2-d1, d1-d3 over row pairs) run on DVE in
    bf16 at the 2x rate, striding row pairs in the padded x grid.
  - U comps (w0+w1+w2 etc. per kj) run once on DVE; per-sample
    modulation is a tensor_scalar_mul by s (or 0.5*s) per partition.
  - Inverse transform = 2 adds + 2 subs on fp32 PSUM per chunk,
    split between DVE and gpsimd(Pool), writing even/odd row pairs
    straight into the output SBUF tile.
Host-side work is layout-only (transpose / reshape / pad / dtype cast).
"""

import numpy as np
import ml_dtypes

B, D_CAT, C_IN, C_OUT, K, H, W = 16, 512, 256, 256, 3, 64, 64
NCORES = 8
BL = B // NCORES          # samples per core (2)
CIT = C_IN // 128         # ci tiles (2)
COT = C_OUT // 128        # co tiles (2)
GW = W + 2                # padded grid width (66)
GH = 68                   # padded grid height (66 used + 2 extra pad rows
                          # so the row-pair rearrange views stay in-bounds)
TIG = 8                   # winograd row-pairs (ti) per chunk
NCH = (H // 2) // TIG     # chunks per (sample, co-tile) (4)
VROWS = H // 2            # ti count (32)
VSZ = 4 * VROWS * GW      # V tile: 4 comps x 32 ti x 66 cols

# packed MLP-param column offsets (per partition). Weights and y ship in bf16
# (pp1 = y + w0, pp2 = w1, pp3 = w2, ordered by first use); biases in fp32.
_PY = 0                       # y^T:   4 k-tiles x BL
_PW0 = _PY + 4 * BL           # w0^T:  4 k-tiles x 256
_P1TOT = _PW0 + 4 * C_IN
_P2TOT = 2 * C_IN             # w1^T
_P3TOT = 2 * C_IN             # w2^T
_NBIAS = 3 * CIT              # b0, b1, b2 per ci-tile (fp32)

_BF16 = ml_dtypes.bfloat16
_COMPILED = None

def _build():
    import concourse.mybir as mybir
    import concourse.tile as tile
    from concourse import bacc

    bf16 = mybir.dt.bfloat16
    f32 = mybir.dt.float32
    Prelu = mybir.ActivationFunctionType.Prelu

    nc = bacc.Bacc("TRN2", target_bir_lowering=False, debug=False, num_devices=NCORES)

    pp1_in = nc.declare_dram_parameter("pp1", [128, _P1TOT], bf16, isOutput=False)
    pp23_in = nc.declare_dram_parameter("pp23", [128, _P2TOT + _P3TOT], bf16, isOutput=False)
    bias_in = nc.declare_dram_parameter("bias", [128, _NBIAS], f32, isOutput=False)
    wf_in = nc.declare_dram_parameter("wf", [CIT, COT, 128, K * K * 128], bf16, isOutput=False)
    xb_in = nc.declare_dram_parameter("xb", [BL, CIT, 128, GH * GW], bf16, isOutput=False)
    out_ext = nc.declare_dram_parameter("out", [BL, COT, 128, H * W], f32, isOutput=True)

    with tile.TileContext(nc) as tc:
        with (
            tc.tile_pool(name="const", bufs=1) as cpool,
            tc.tile_pool(name="xpad", bufs=4) as padpool,
            tc.tile_pool(name="vt", bufs=1) as vpool,
            tc.tile_pool(name="osb", bufs=5) as opool,
            tc.tile_pool(name="scr", bufs=2) as spool,
            tc.tile_pool(name="cpsum", bufs=8, space="PSUM") as cpsum,
        ):
            # warm the scalar-engine activation table before the params land
            warm = cpool.tile([128, 1], f32)
            nc.vector.memset(warm[:], 0.0)
            nc.scalar.activation(warm[:], warm[:], Prelu, bias=warm[:], scale=1.0, alpha=0.01)

            # ---- DMAs: params on the sync HWDGE queue, conv weights on the
            # act HWDGE queue, x grids on the gpsimd SWDGE queue ----
            # The cost model serializes all DMA transfers on one engine, so
            # kick order IS landing order: params and co0 weights first, then
            # sample-0 grids in half-tile slices (top halves cover the first
            # two chunk groups), co1 weights, then sample-1 grids.
            GS1 = 18 * GW      # grid rows 0..17: all that chunk group 0 reads
            GS2 = 34 * GW      # rows 18..33: group 1; rest: groups 2..3
            grids = {}
            for b in range(BL):
                for ci_t in range(CIT):
                    t = padpool.tile([128, GH * GW], bf16)
                    grids[(b, ci_t)] = t
            pp1_sb = cpool.tile([128, _P1TOT], bf16)
            nc.sync.dma_start(pp1_sb[:], pp1_in[:])
            bias_sb = cpool.tile([128, _NBIAS], f32)
            nc.sync.dma_start(bias_sb[:], bias_in[:])
            pp23_sb = cpool.tile([128, _P2TOT + _P3TOT], bf16)
            nc.sync.dma_start(pp23_sb[:], pp23_in[:])

            wf_sbs = {}

            def wf_dma(ci_t, co_t):
                t = cpool.tile(
                    [128, K * K * 128], bf16, tag=f"wf{ci_t}{co_t}", name=f"wf{ci_t}{co_t}"
                )
                nc.sync.dma_start(t[:], wf_in[ci_t, co_t])
                wf_sbs[(ci_t, co_t)] = t

            def grid_dma(b, ci_t, lo, hi):
                nc.sync.dma_start(grids[(b, ci_t)][:, lo:hi], xb_in[b, ci_t][:, lo:hi])

            wf_dma(0, 0)
            grid_dma(0, 0, 0, GS1)
            grid_dma(0, 1, 0, GS1)
            wf_dma(1, 0)
            grid_dma(0, 0, GS1, GS2)
            grid_dma(0, 1, GS1, GS2)
            grid_dma(0, 0, GS2, GH * GW)
            grid_dma(0, 1, GS2, GH * GW)
            wf_dma(0, 1)
            wf_dma(1, 1)
            grid_dma(1, 0, 0, GS2)
            grid_dma(1, 1, 0, GS2)
            grid_dma(1, 0, GS2, GH * GW)
            grid_dma(1, 1, GS2, GH * GW)

            # ---- style MLP (fp32): s^T per ci-tile in SBUF ----
            def mlp_layer(rhs_of_kt, kts, w_sb, w_base, bias_ap, out_sb):
                for ct in range(CIT):
                    mps = cpsum.tile([128, TIG * W], f32, tag="cps")
                    for kt in range(kts):
                        nc.tensor.matmul(
                            mps[:, :BL],
                            w_sb[:, w_base + kt * C_IN + ct * 128 :][:, :128],
                            rhs_of_kt(kt),
                            start=(kt == 0),
                            stop=(kt == kts - 1),
                        )
                    nc.scalar.activation(
                        out_sb[:, ct * BL : (ct + 1) * BL],
                        mps[:, :BL],
                        Prelu,
                        bias=bias_ap(ct),
                        scale=1.0,
                        alpha=0.01,
                    )

            s0_sb = cpool.tile([128, CIT * BL], bf16)
            s1_sb = cpool.tile([128, CIT * BL], bf16)
            s_sb = cpool.tile([128, CIT * BL], f32)
            mlp_layer(
                lambda kt: pp1_sb[:, _PY + kt * BL : _PY + (kt + 1) * BL],
                4, pp1_sb, _PW0,
                lambda ct: bias_sb[:, ct : ct + 1],
                s0_sb,
            )
            mlp_layer(
                lambda kt: s0_sb[:, kt * BL : (kt + 1) * BL],
                2, pp23_sb, 0,
                lambda ct: bias_sb[:, CIT + ct : CIT + ct + 1],
                s1_sb,
            )
            mlp_layer(
                lambda kt: s1_sb[:, kt * BL : (kt + 1) * BL],
                2, pp23_sb, _P2TOT,
                lambda ct: bias_sb[:, 2 * CIT + ct : 2 * CIT + ct + 1],
                s_sb,
            )

            # ---- V input transform (DVE, bf16 2x): per (sample, ci_t) tile
            # [128, 4 comps x 32 ti x 66 cols]. Row-pair views of the padded
            # grid: dp0[t] = grid row 2t, dp1[t] = grid row 2t+1. ----
            vts = {}

            def v_views(b, ci_t):
                g = grids[(b, ci_t)]
                dp0 = g[:, : 34 * 2 * GW].rearrange("p (t c) -> p t c", c=2 * GW)
                dp1 = g[:, GW : GW + 33 * 2 * GW].rearrange("p (t c) -> p t c", c=2 * GW)
                return dp0, dp1

            def emit_v(b, ci_t, n0, n1, eng):
                # comps for ti in [8*n0, 8*n1): V0=d0-d2 V1=d1+d2 V2=d2-d1 V3=d1-d3
                dp0, dp1 = v_views(b, ci_t)
                v = vts[(b, ci_t)]
                t0, t1 = TIG * n0, TIG * n1
                d0 = dp0[:, t0:t1, :GW]
                d1 = dp1[:, t0:t1, :GW]
                d2 = dp0[:, t0 + 1 : t1 + 1, :GW]
                d3 = dp1[:, t0 + 1 : t1 + 1, :GW]
                comps = [(d0, d2, "sub"), (d1, d2, "add"), (d2, d1, "sub"), (d1, d3, "sub")]
                for a in (0, 3, 1, 2):
                    x0, x1, op = comps[a]
                    dst = v[:, a * VROWS * GW + t0 * GW : a * VROWS * GW + t1 * GW]
                    getattr(eng, f"tensor_{op}")(dst, x0, x1)

            for b in range(BL):
                for ci_t in range(CIT):
                    vts[(b, ci_t)] = vpool.tile(
                        [128, VSZ], bf16, name=f"vt{b}{ci_t}", tag=f"vt{b}{ci_t}"
                    )


            # ---- U weight transform + per-sample modulation ----
            # wf cols: (ki*3+kj)*128 + co. U0 = w_ki0, U3 = w_ki2,
            # U1 = 0.5*(w0+w1+w2), U2 = 0.5*(w0-w1+w2); the 0.5 goes into the
            # modulation scalar so U12u holds unhalved sums in bf16.
            u12u = {}

            def emit_uprep(ci_t, co_t):
                wfv = wf_sbs[(ci_t, co_t)]
                ts = spool.tile([128, 384], bf16, tag=f"uscr{co_t}")
                u = cpool.tile([128, 768], bf16, tag=f"u12_{ci_t}{co_t}")
                nc.vector.tensor_add(ts[:], wfv[:, 0:384], wfv[:, 768:1152])
                nc.vector.tensor_add(u[:, 0:384], ts[:], wfv[:, 384:768])
                nc.vector.tensor_sub(u[:, 384:768], ts[:], wfv[:, 384:768])
                u12u[(ci_t, co_t)] = u

            u_mods = {}

            def emit_umod(b, ci_t, co_t):
                wfv = wf_sbs[(ci_t, co_t)]
                um = cpool.tile([128, 4 * 384], bf16, tag=f"um{b}{ci_t}{co_t}", name="um")
                s_col = s_sb[:, ci_t * BL + b : ci_t * BL + b + 1]
                sh_col = s_half[:, ci_t * BL + b : ci_t * BL + b + 1]
                nc.scalar.mul(um[:, 0:384], wfv[:, 0:384], s_col)
                nc.scalar.mul(um[:, 1152:1536], wfv[:, 768:1152], s_col)
                nc.scalar.mul(um[:, 384:1152], u12u[(ci_t, co_t)][:], sh_col)
                u_mods[(b, ci_t, co_t)] = um

            s_half = cpool.tile([128, CIT * BL], f32)
            nc.scalar.mul(s_half[:], s_sb[:], 0.5)
            # DVE: co0 weight combos + the first two sample-0 chunk groups;
            # co1 combos and the rest of sample 0 slot between chunk inverses
            emit_uprep(0, 0)
            emit_v(0, 0, 0, 1, nc.vector)
            emit_v(0, 1, 0, 1, nc.vector)
            emit_uprep(1, 0)
            emit_v(0, 0, 1, 2, nc.vector)
            emit_v(0, 1, 1, 2, nc.vector)
            # Act: co0 sample-0 modulated U tiles (idle after the MLP); the
            # rest slot in between chunk PSUM drains below
            for ci_t in range(CIT):
                emit_umod(0, ci_t, 0)
            # Pool: sample-1 ci0 input transform (slow but far off critical
            # path); ci1 runs on DVE between mid-stream chunk inverses
            emit_v(1, 0, 0, NCH, nc.gpsimd)

            # ---- conv chunks: per (sample, co_t, n): 4 PSUM comp tiles
            # M_a [128, 8 ti x 64], 24 accumulating matmuls, then the inverse
            # transform writes even/odd output rows and the chunk DMAs out ----
            def conv_chunk(b, co_t, n, inv_eng, tis=None):
                # tis=(t0, len) overrides the ti range of the group
                t0, tig = tis if tis is not None else (TIG * n, TIG)
                pss = [
                    cpsum.tile([128, TIG * W], f32, name=f"cps_{b}_{co_t}_{t0}_{a}", tag="cps")
                    for a in range(4)
                ]
                q = 0
                for ci_t in range(CIT):
                    um = u_mods[(b, ci_t, co_t)]
                    v = vts[(b, ci_t)]
                    for a in (0, 3, 1, 2):
                        for kj in range(K):
                            rhs = (
                                v[:, a * VROWS * GW + t0 * GW :][:, : tig * GW]
                                .rearrange("p (t c) -> p t c", c=GW)[:, :, kj : kj + W]
                            )
                            nc.tensor.matmul(
                                pss[a][:, : tig * W],
                                um[:, a * 384 + kj * 128 : a * 384 + (kj + 1) * 128],
                                rhs,
                                start=(ci_t == 0 and q % K == 0),
                                stop=(ci_t == CIT - 1 and q % K == K - 1),
                            )
                            q += 1
                # inverse: even = M0 + M1' + M2', odd = M1' - M2' - M3.
                # DVE tensor_tensor may read only one PSUM operand, so the
                # (otherwise idle) act engine drains M1' to SBUF first.
                o_sb = opool.tile([128, 2 * TIG * W], f32, tag="osb")
                orows = o_sb[:, : 2 * tig * W].rearrange("p (t c) -> p t c", c=2 * W)
                c1 = spool.tile([128, TIG * W], f32, tag="c1scr", name="c1")
                nc.scalar.copy(c1[:, : tig * W], pss[1][:, : tig * W])
                sc1 = spool.tile([128, TIG * W], f32, tag="iscr1", name="sc1")
                sc2 = spool.tile([128, TIG * W], f32, tag="iscr2", name="sc2")
                cw = tig * W
                inv_eng.tensor_add(sc1[:, :cw], c1[:, :cw], pss[2][:, :cw])
                inv_eng.tensor_add(orows[:, :, 0:W], sc1[:, :cw].rearrange("p (t c) -> p t c", c=W), pss[0][:, :cw].rearrange("p (t c) -> p t c", c=W))
                inv_eng.tensor_sub(sc2[:, :cw], c1[:, :cw], pss[2][:, :cw])
                inv_eng.tensor_sub(orows[:, :, W : 2 * W], sc2[:, :cw].rearrange("p (t c) -> p t c", c=W), pss[3][:, :cw].rearrange("p (t c) -> p t c", c=W))
                nc.sync.dma_start(
                    out_ext[b, co_t][:, 2 * t0 * W : 2 * (t0 + tig) * W],
                    o_sb[:, : 2 * tig * W],
                )

            u_negs = {}

            def emit_uneg(ci_t):
                un = cpool.tile([128, 768], bf16, tag=f"uneg{ci_t}", name="un")
                nc.vector.tensor_scalar_mul(
                    un[:], u_mods[(BL - 1, ci_t, COT - 1)][:, 768:1536], -1.0
                )
                u_negs[ci_t] = un

            def folded_piece(b, co_t, t0, tig):
                pse = cpsum.tile([128, TIG * W], f32, name="pse", tag="cps")
                pso = cpsum.tile([128, TIG * W], f32, name="pso", tag="cps")
                qq = {0: 0, 1: 0}
                for ci_t in range(CIT):
                    um = u_mods[(b, ci_t, co_t)]
                    un = u_negs[ci_t]
                    v = vts[(b, ci_t)]
                    for kj in range(K):
                        rhss = {}
                        for a in range(4):
                            rhss[a] = (
                                v[:, a * VROWS * GW + t0 * GW :][:, : tig * GW]
                                .rearrange("p (t c) -> p t c", c=GW)[:, :, kj : kj + W]
                            )
                        for pi, (ps, combos) in enumerate((
                            (pse, [(0, um, 0), (1, um, 1), (2, um, 2)]),
                            (pso, [(1, um, 1), (2, un, 0), (3, un, 1)]),
                        )):
                            for a, usrc, sl in combos:
                                nc.tensor.matmul(
                                    ps[:, : tig * W],
                                    usrc[:, sl * 384 + kj * 128 : sl * 384 + (kj + 1) * 128],
                                    rhss[a],
                                    start=(qq[pi] == 0),
                                    stop=(qq[pi] == 2 * K * 3 - 1),
                                )
                                qq[pi] += 1
                o_sb = opool.tile([128, 2 * TIG * W], f32, tag="osb", name="osbf")
                orows = o_sb[:, : 2 * tig * W].rearrange("p (t c) -> p t c", c=2 * W)
                nc.scalar.copy(orows[:, :, 0:W], pse[:, : tig * W].rearrange("p (t c) -> p t c", c=W))
                nc.scalar.copy(orows[:, :, W : 2 * W], pso[:, : tig * W].rearrange("p (t c) -> p t c", c=W))
                nc.sync.dma_start(
                    out_ext[b, co_t][:, 2 * t0 * W : 2 * (t0 + tig) * W],
                    o_sb[:, : 2 * tig * W],
                )

            umod1 = [(ci_t, co_t) for co_t in range(COT) for ci_t in range(CIT)]
            chunk_idx = 0
            for b in range(BL):
                for co_t in range(COT):
                    for n in range(NCH):
                        last = b == BL - 1 and co_t == COT - 1 and n == NCH - 1
                        if last:
                            # split the final chunk so its drain/store tail
                            # overlaps the trailing matmuls; the very last
                            # piece folds the inverse transform into the PE
                            # accumulation (1.5x matmuls for that piece) and
                            # stores via idle-engine copies, so almost
                            # nothing remains after the final matmul
                            conv_chunk(b, co_t, n, nc.vector, tis=(TIG * n, 4))
                            conv_chunk(b, co_t, n, nc.vector, tis=(TIG * n + 4, 2))
                            folded_piece(b, co_t, NCH * TIG - 2, 2)
                        else:
                            conv_chunk(b, co_t, n, nc.vector)
                        if chunk_idx == 0:
                            for ci_t in range(CIT):
                                emit_uprep(ci_t, 1)
                            emit_v(0, 0, 2, 3, nc.vector)
                            emit_v(0, 1, 2, 3, nc.vector)
                            emit_umod(0, 0, 1)
                        elif chunk_idx == 1:
                            emit_v(0, 0, 3, 4, nc.vector)
                            emit_v(0, 1, 3, 4, nc.vector)
                            emit_umod(0, 1, 1)
                        elif 3 <= chunk_idx <= 6:
                            ci_t, co_t1 = umod1[chunk_idx - 3]
                            emit_umod(1, ci_t, co_t1)
                            emit_v(1, 1, chunk_idx - 3, chunk_idx - 2, nc.vector)
                        elif chunk_idx == 7:
                            for ci_t in range(CIT):
                                emit_uneg(ci_t)
                        chunk_idx += 1

    nc.compile()
    return nc


def _get_nc():
    global _COMPILED
    if _COMPILED is None:
        _COMPILED = _build()
    return _COMPILED


def _prep_in_maps(x, y, w0, b0, w1, b1, w2, b2, conv_w):
    x = np.ascontiguousarray(x, dtype=np.float32)
    y = np.ascontiguousarray(y, dtype=np.float32)

    # packed per-core-invariant params (bf16 weights, fp32 biases)
    pp1_shared = np.empty((128, _P1TOT), dtype=_BF16)
    pp1_shared[:, _PW0 : _PW0 + 4 * C_IN] = (
        w0.astype(np.float32).T.reshape(4, 128, C_IN).transpose(1, 0, 2).reshape(128, 4 * C_IN)
    ).astype(_BF16)
    pp23 = np.empty((128, _P2TOT + _P3TOT), dtype=_BF16)
    pp23[:, :_P2TOT] = (
        w1.astype(np.float32).T.reshape(2, 128, C_IN).transpose(1, 0, 2).reshape(128, 2 * C_IN)
    ).astype(_BF16)
    pp23[:, _P2TOT:] = (
        w2.astype(np.float32).T.reshape(2, 128, C_IN).transpose(1, 0, 2).reshape(128, 2 * C_IN)
    ).astype(_BF16)
    bias = np.empty((128, _NBIAS), dtype=np.float32)
    for i, bb in enumerate((b0, b1, b2)):
        bias[:, i * CIT : (i + 1) * CIT] = bb.astype(np.float32).reshape(CIT, 128).T

    # conv weights: (co_t, co, ci_t, ci, ki, kj) -> (ci_t, co_t, ci, (ki kj) co)
    wf = np.ascontiguousarray(
        conv_w.astype(np.float32)
        .reshape(COT, 128, CIT, 128, K, K)
        .transpose(2, 0, 3, 4, 5, 1)
        .reshape(CIT, COT, 128, K * K * 128)
    ).astype(_BF16)

    xb_all = np.zeros((B, CIT, 128, GH, GW), dtype=_BF16)
    xb_all[:, :, :, 1 : H + 1, 1 : W + 1] = x.reshape(B, CIT, 128, H, W)
    xb_all = xb_all.reshape(B, CIT, 128, GH * GW)

    in_maps = []
    for c in range(NCORES):
        sl = slice(c * BL, (c + 1) * BL)
        pp1 = pp1_shared.copy()
        pp1[:, _PY : _PY + 4 * BL] = (
            y[sl].T.reshape(4, 128, BL).transpose(1, 0, 2).reshape(128, 4 * BL)
        ).astype(_BF16)
        in_maps.append(
            {
                "pp1": pp1,
                "pp23": pp23,
                "bias": bias,
                "wf": wf,
                "xb": np.ascontiguousarray(xb_all[sl]),
            }
        )
    return in_maps


def _run(in_maps, trace=False):
    from concourse.bass_utils import run_bass_kernel_spmd

    nc = _get_nc()
    res = run_bass_kernel_spmd(nc, in_maps, list(range(NCORES)), trace=trace)
    out = np.concatenate(
        [res.results[c]["out"].reshape(BL, C_OUT, H, W) for c in range(NCORES)], axis=0
    ).astype(np.float32, copy=False)
    return out, res


def kernel(x, y, w0, b0, w1, b1, w2, b2, conv_w):
    in_maps = _prep_in_maps(x, y, w0, b0, w1, b1, w2, b2, conv_w)
    out, _ = _run(in_maps, trace=False)
    return out
